# revision 37
# baseline (speedup 1.0000x reference)
"""Trainium2 Bass kernel for nn_AttentionLayer (scatter_memory).

Reference math (per batch b):
    heatmap[k,y,x] += vis_k at (y_k, x_k)              # scatter, <=19 nonzero px
    kp_feat = conv1x1_K->K(heatmap)                    # kp_proj_w/b
    img_proj = img_fc(img)                             # C x C linear over pixels
    kp_proj  = kp_fc(kp_feat)                          # K -> C linear
    combined = tanh(img_proj + kp_proj)
    scores   = sigmoid(attn_fc(combined))              # per-pixel scalar
    out      = img * scores

Because the heatmap has at most K=19 nonzero pixels (one-hot rows), the whole
keypoint path folds to a rank-19 correction of the big matmul:
    pre_tanh[o,s] = sum_c W[o,c] img[c,s] + sum_j M[o,j] onehot[j,s] + bias[o]
with host-folded constants:
    W    = img_fc_w                     (used transposed as lhsT)
    M    = kp_fc_w @ kp_proj_w          [C,K]
    bias = img_fc_b + kp_fc_w @ kp_proj_b + kp_fc_b
    onehot[j,s] = (vis_j>0) * [s == y_j*W + x_j]       built on device:
index math on DVE (exact fp32, robust floor), then each [19, 1024] one-hot
chunk is materialized in SBUF by one fused DVE op, (iota == s_j - 1024q)*vis,
pipelined one pair ahead of the matmuls that consume it. Keypoint collisions
sum in PSUM naturally.

The attention reduction z[s] = sum_o attn_w[o] combined[o,s] runs as a matmul
whose lhsT is attn_w replicated across 128 columns, so the PSUM result
[128, 512] already holds z broadcast across all partitions -- sigmoid and the
final elementwise multiply need no partition-broadcast step.

Matmuls run in bf16 (full PE rate, FWL weight loads, HAM warms up). The PE
reads the image as a TRUNCATED-bf16 strided view of the fp32 tiles (top two
bytes of each f32 via bitcast + stride-2 AP) -- no cast ops, no extra DMA.
The final multiply uses the original fp32 image tiles, so output error comes
only through `scores` (~1.3e-3 relative). Loads issue on the sync HWDGE ring
and stores on the scalar HWDGE ring (independent FIFOs).

Sharding: pure data parallelism, batch b -> NeuronCore b (weights replicated).
"""

import sys
from contextlib import ExitStack

import numpy as np

sys.path.insert(0, "/opt/trn_rl_repo")

import concourse.bacc as bacc
import concourse.bass as bass
import concourse.mybir as mybir
import concourse.tile as tile
from concourse.bass_utils import run_bass_kernel_spmd

F32 = mybir.dt.float32
BF16 = mybir.dt.bfloat16
I32 = mybir.dt.int32
AF = mybir.ActivationFunctionType
OP = mybir.AluOpType

B, C, H, W, K = 8, 256, 128, 128, 19
S = H * W                  # 16384 pixels
ST = 512                   # pixel tile (one PSUM bank)
NT = S // ST               # 32 tiles
_CACHE: dict = {}


def _emit(tc: tile.TileContext, io: dict):
    nc = tc.nc
    img, kp, wt, mt, bias, arep, ab, out = (
        io["img"], io["kp"], io["wt"], io["mt"],
        io["bias"], io["arep"], io["ab"], io["out"],
    )
    with ExitStack() as ctx:
        consts = ctx.enter_context(tc.tile_pool(name="consts", bufs=1))
        small = ctx.enter_context(tc.tile_pool(name="small", bufs=1))
        imgp = ctx.enter_context(tc.tile_pool(name="imgp", bufs=6))
        combp = ctx.enter_context(tc.tile_pool(name="combp", bufs=6))
        scorep = ctx.enter_context(tc.tile_pool(name="scorep", bufs=4))
        outp = ctx.enter_context(tc.tile_pool(name="outp", bufs=4))
        psum = ctx.enter_context(tc.tile_pool(name="psum", bufs=2, space="PSUM"))
        ohp = ctx.enter_context(tc.tile_pool(name="ohp", bufs=3))

        # ---- constants into SBUF (weights pre-cast to bf16 on host) ----
        wt0 = consts.tile([128, C], BF16)          # W^T rows c=0..127
        wt1 = consts.tile([128, C], BF16)          # W^T rows c=128..255
        nc.sync.dma_start(wt0[:], wt[0:128, :])
        nc.sync.dma_start(wt1[:], wt[128:256, :])
        mts = consts.tile([K, C], BF16)            # M^T [19, 256]
        nc.sync.dma_start(mts[:], mt[:, :])
        ar0 = consts.tile([128, 128], BF16)        # attn_w replicated, o=0..127
        ar1 = consts.tile([128, 128], BF16)
        nc.sync.dma_start(ar0[:], arep[0:128, :])
        nc.sync.dma_start(ar1[:], arep[128:256, :])
        kpt = small.tile([K, 3], F32)
        nc.scalar.dma_start(kpt[:], kp[:, :])
        b0 = consts.tile([128, 1], F32)
        b1 = consts.tile([128, 1], F32)
        nc.scalar.dma_start(b0[:], bias[0:128, :])
        nc.scalar.dma_start(b1[:], bias[128:256, :])
        abt = consts.tile([128, 1], F32)
        nc.scalar.dma_start(abt[:], ab[:, :])

        # ---- build one-hot [K, S] on device ----
        # index math (all [19,1], exact fp32; matches reference:
        # x = int(clip(kx/128, 0, 127)), s = y*128 + x)

        def floor_clipped(col):
            v = small.tile([K, 1], F32, name=f"v{col}")
            nc.vector.tensor_scalar(v[:], kpt[:, col:col + 1], 1.0 / 128.0, None, OP.mult)
            nc.vector.tensor_scalar(v[:], v[:], 127.0, 0.0, OP.min, OP.max)
            vi = small.tile([K, 1], I32, name=f"vi{col}")
            nc.vector.tensor_copy(vi[:], v[:])        # any rounding mode works:
            vf = small.tile([K, 1], F32, name=f"vf{col}")
            nc.vector.tensor_copy(vf[:], vi[:])       # fixed up below
            gt = small.tile([K, 1], F32, name=f"gt{col}")
            nc.vector.tensor_tensor(gt[:], vf[:], v[:], op=OP.is_gt)
            nc.vector.tensor_tensor(vf[:], vf[:], gt[:], op=OP.subtract)
            return vf

        xf = floor_clipped(0)
        yf = floor_clipped(1)
        sf = small.tile([K, 1], F32)                  # pixel index y*128+x
        nc.vector.tensor_scalar(sf[:], yf[:], 128.0, xf[:, 0:1], OP.mult, OP.add)
        vis = small.tile([K, 1], F32)                 # 1.0 where visible
        nc.vector.tensor_scalar(vis[:], kpt[:, 2:3], 0.0, None, OP.is_gt)
        ioti = small.tile([K, 1024], I32)             # 0..1023 along free dim
        nc.gpsimd.iota(ioti[:], pattern=[[1, 1024]], base=0, channel_multiplier=0)
        iotf = small.tile([K, 1024], F32)
        nc.vector.tensor_copy(iotf[:], ioti[:])

        # one-hot chunk for pair q (1024 px): (iota == s - 1024q) * vis, one
        # fused DVE op per chunk; emitted one pair ahead of its consumers.
        def make_chunk(q):
            cv = small.tile([K, 1], F32, name=f"cv{q}")
            nc.vector.tensor_scalar(cv[:], sf[:], float(1024 * q), None, OP.subtract)
            oc = ohp.tile([K, 1024], BF16, tag="oh")
            nc.vector.tensor_scalar(oc[:], iotf[:], cv[:, 0:1], vis[:, 0:1],
                                    OP.is_equal, OP.mult)
            return oc

        # ---- main pixel loop: pairs of 512-px tiles (1024 px per DMA) ----
        # Attention matmuls + sigmoid + final mul run TWO pairs BEHIND the
        # main matmuls, so the PE stream never waits on a tanh issued in the
        # same iteration (keeps PE dense -> HAM stays warm).
        PT = 2 * ST
        NP = NT // 2
        from collections import deque
        pending = deque()          # attn stage runs TWO pairs behind
        DEPTH = 2
        next_chunk = make_chunk(0)

        def drain(dfr):
            sc, dim0, dim1, dslp, halves = dfr
            (dcb0a, dcb1a, dhs_a), (dcb0b, dcb1b, dhs_b) = halves
            pza = psum.tile([128, ST], F32, tag="psz", name="pza")
            pzb = psum.tile([128, ST], F32, tag="psz", name="pzb")
            nc.tensor.matmul(out=pza[:], lhsT=ar0[:], rhs=dcb0a[:], start=True, stop=False)
            nc.tensor.matmul(out=pzb[:], lhsT=ar0[:], rhs=dcb0b[:], start=True, stop=False)
            nc.tensor.matmul(out=pza[:], lhsT=ar1[:], rhs=dcb1a[:], start=False, stop=True)
            nc.tensor.matmul(out=pzb[:], lhsT=ar1[:], rhs=dcb1b[:], start=False, stop=True)
            nc.scalar.activation(sc[:, dhs_a], pza[:], AF.Sigmoid, bias=abt[:, 0:1])
            nc.scalar.activation(sc[:, dhs_b], pzb[:], AF.Sigmoid, bias=abt[:, 0:1])
            o0 = outp.tile([128, PT], F32, tag="o0")
            o1 = outp.tile([128, PT], F32, tag="o1")
            nc.vector.tensor_mul(o0[:], dim0[:], sc[:])
            nc.vector.tensor_mul(o1[:], dim1[:], sc[:])
            nc.scalar.dma_start(out[0:128, dslp], o0[:])
            nc.scalar.dma_start(out[128:256, dslp], o1[:])

        for p in range(NP):
            slp = bass.ts(p, PT)
            im0 = imgp.tile([128, PT], F32, tag="im0")
            im1 = imgp.tile([128, PT], F32, tag="im1")
            nc.sync.dma_start(im0[:], img[0:128, slp])
            nc.sync.dma_start(im1[:], img[128:256, slp])
            # truncated-bf16 views of the fp32 tiles (top 2 bytes of each f32)
            ib0 = im0[:].bitcast(BF16)[:, 1::2]
            ib1 = im1[:].bitcast(BF16)[:, 1::2]

            sc = scorep.tile([128, PT], F32, tag="sc")
            oh = next_chunk
            if p + 1 < NP:
                next_chunk = make_chunk(p + 1)
            if len(pending) >= DEPTH:
                drain(pending.popleft())
            if p == NP - 1 and pending:
                drain(pending.popleft())   # pull the tail stage into the loop
            # same stationary weight used for both halves back-to-back
            hs0, hs1 = bass.ts(0, ST), bass.ts(1, ST)
            pA0 = psum.tile([128, ST], F32, tag="ps0", bufs=3)
            pB0 = psum.tile([128, ST], F32, tag="ps0", bufs=3, name="pB0")
            pA1 = psum.tile([128, ST], F32, tag="ps1", bufs=3)
            pB1 = psum.tile([128, ST], F32, tag="ps1", bufs=3, name="pB1")
            nc.tensor.matmul(out=pA0[:], lhsT=wt0[:, 0:128], rhs=ib0[:, hs0], start=True, stop=False)
            nc.tensor.matmul(out=pB0[:], lhsT=wt0[:, 0:128], rhs=ib0[:, hs1], start=True, stop=False)
            nc.tensor.matmul(out=pA0[:], lhsT=wt1[:, 0:128], rhs=ib1[:, hs0], start=False, stop=False)
            nc.tensor.matmul(out=pB0[:], lhsT=wt1[:, 0:128], rhs=ib1[:, hs1], start=False, stop=False)
            nc.tensor.matmul(out=pA0[:], lhsT=mts[:, 0:128], rhs=oh[:, hs0], start=False, stop=True)
            nc.tensor.matmul(out=pB0[:], lhsT=mts[:, 0:128], rhs=oh[:, hs1], start=False, stop=True)
            nc.tensor.matmul(out=pA1[:], lhsT=wt0[:, 128:256], rhs=ib0[:, hs0], start=True, stop=False)
            nc.tensor.matmul(out=pB1[:], lhsT=wt0[:, 128:256], rhs=ib0[:, hs1], start=True, stop=False)
            nc.tensor.matmul(out=pA1[:], lhsT=wt1[:, 128:256], rhs=ib1[:, hs0], start=False, stop=False)
            nc.tensor.matmul(out=pB1[:], lhsT=wt1[:, 128:256], rhs=ib1[:, hs1], start=False, stop=False)
            nc.tensor.matmul(out=pA1[:], lhsT=mts[:, 128:256], rhs=oh[:, hs0], start=False, stop=True)
            nc.tensor.matmul(out=pB1[:], lhsT=mts[:, 128:256], rhs=oh[:, hs1], start=False, stop=True)

            halves = []
            for h, (q0, q1) in enumerate(((pA0, pA1), (pB0, pB1))):
                cb0 = combp.tile([128, ST], BF16, tag="cb0")
                cb1 = combp.tile([128, ST], BF16, tag="cb1")
                nc.scalar.activation(cb0[:], q0[:], AF.Tanh, bias=b0[:, 0:1])
                nc.scalar.activation(cb1[:], q1[:], AF.Tanh, bias=b1[:, 0:1])
                halves.append((cb0, cb1, bass.ts(h, ST)))

            pending.append((sc, im0, im1, slp, halves))

        while pending:
            drain(pending.popleft())

def _build():
    if "nc" in _CACHE:
        return _CACHE["nc"]
    nc = bacc.Bacc("TRN2", target_bir_lowering=False, debug=False)
    io = {
        "img": nc.dram_tensor("img", [C, S], F32, kind="ExternalInput").ap(),
        "kp": nc.dram_tensor("kp", [K, 3], F32, kind="ExternalInput").ap(),
        "wt": nc.dram_tensor("wt", [C, C], BF16, kind="ExternalInput").ap(),
        "mt": nc.dram_tensor("mt", [K, C], BF16, kind="ExternalInput").ap(),
        "bias": nc.dram_tensor("bias", [C, 1], F32, kind="ExternalInput").ap(),
        "arep": nc.dram_tensor("arep", [C, 128], BF16, kind="ExternalInput").ap(),
        "ab": nc.dram_tensor("ab", [128, 1], F32, kind="ExternalInput").ap(),
        "out": nc.dram_tensor("out", [C, S], F32, kind="ExternalOutput").ap(),
    }
    with tile.TileContext(nc) as tc:
        _emit(tc, io)
    nc.compile()
    _CACHE["nc"] = nc
    return nc


def _in_maps(image_features, keypoint_features, img_fc_w, img_fc_b,
             kp_proj_w, kp_proj_b, kp_fc_w, kp_fc_b, attn_fc_w, attn_fc_b):
    import ml_dtypes

    f = lambda a: np.ascontiguousarray(np.asarray(a, dtype=np.float32))
    bf = lambda a: np.ascontiguousarray(np.asarray(a, dtype=np.float32).astype(ml_dtypes.bfloat16))
    img_fc_w, img_fc_b = f(img_fc_w), f(img_fc_b)
    kp_proj_w, kp_proj_b = f(kp_proj_w), f(kp_proj_b)
    kp_fc_w, kp_fc_b = f(kp_fc_w), f(kp_fc_b)
    attn_fc_w, attn_fc_b = f(attn_fc_w), f(attn_fc_b)

    wt = bf(img_fc_w.T)                                         # [C, C]
    mt = bf((kp_fc_w @ kp_proj_w).T)                            # [K, C]
    bias = f((img_fc_b + kp_fc_w @ kp_proj_b + kp_fc_b).reshape(C, 1))
    arep = bf(np.repeat(attn_fc_w.reshape(C, 1), 128, axis=1))
    ab = np.full((128, 1), float(attn_fc_b.reshape(-1)[0]), np.float32)

    imgs = f(image_features).reshape(B, C, S)
    kps = f(keypoint_features)
    return [
        {
            "img": np.ascontiguousarray(imgs[b]),
            "kp": np.ascontiguousarray(kps[b]),
            "wt": wt, "mt": mt, "bias": bias, "arep": arep, "ab": ab,
        }
        for b in range(B)
    ]


def _run(in_maps, trace=False, tmpdir=None):
    nc = _build()
    return run_bass_kernel_spmd(
        nc, in_maps, core_ids=list(range(B)), trace=trace, tmpdir=tmpdir
    )


def kernel(**inputs) -> np.ndarray:
    res = _run(_in_maps(**inputs))
    return np.stack([res.results[b]["out"].reshape(C, H, W) for b in range(B)])


def _enable_axon_ntff_hook():
    """Recreate the missing antenv.axon_hooks module and register the NTFF
    profile hook (what trn_boot would do if the image shipped axon_hooks).
    Local profiling only; kernel() never calls this."""
    import types

    if "antenv.axon_hooks" in sys.modules:
        return
    mod = types.ModuleType("antenv.axon_hooks")
    state = {"hook": None}
    mod.set_axon_ntff_profile_hook = lambda h: state.__setitem__("hook", h)
    mod.get_axon_ntff_profile_hook = lambda: state["hook"]
    sys.modules["antenv.axon_hooks"] = mod
    import antenv

    antenv.axon_hooks = mod
    from trn_agent_boot.trn_boot import _ntff_profile_via_ctypes

    mod.set_axon_ntff_profile_hook(_ntff_profile_via_ctypes("/opt/axon/libaxon_pjrt.so"))
    # keep artifacts local -- no bucket in this container
    import concourse.bass_utils as bu

    bu.upload_artifacts = lambda tmpdir: tmpdir


def kernel_traced(**inputs):
    """Like kernel() but profiles: returns (out, exec_time_ns, tmpdir)."""
    import tempfile

    _enable_axon_ntff_hook()
    tmpdir = tempfile.mkdtemp(prefix="bass_trace_")
    res = _run(_in_maps(**inputs), trace=True, tmpdir=tmpdir)
    out = np.stack([res.results[b]["out"].reshape(C, H, W) for b in range(B)])
    return out, res.exec_time_ns, tmpdir


# revision 38
# speedup vs baseline: 1.0474x; 1.0474x over previous
"""Trainium2 Bass kernel for nn_AttentionLayer (scatter_memory).

Reference math (per batch b):
    heatmap[k,y,x] += vis_k at (y_k, x_k)              # scatter, <=19 nonzero px
    kp_feat = conv1x1_K->K(heatmap)                    # kp_proj_w/b
    img_proj = img_fc(img)                             # C x C linear over pixels
    kp_proj  = kp_fc(kp_feat)                          # K -> C linear
    combined = tanh(img_proj + kp_proj)
    scores   = sigmoid(attn_fc(combined))              # per-pixel scalar
    out      = img * scores

Because the heatmap has at most K=19 nonzero pixels (one-hot rows), the whole
keypoint path folds to a rank-19 correction of the big matmul:
    pre_tanh[o,s] = sum_c W[o,c] img[c,s] + sum_j M[o,j] onehot[j,s] + bias[o]
with host-folded constants:
    W    = img_fc_w                     (used transposed as lhsT)
    M    = kp_fc_w @ kp_proj_w          [C,K]
    bias = img_fc_b + kp_fc_w @ kp_proj_b + kp_fc_b
    onehot[j,s] = (vis_j>0) * [s == y_j*W + x_j]       built on device:
index math on DVE (exact fp32, robust floor), then each [19, 1024] one-hot
chunk is materialized in SBUF by one fused DVE op, (iota == s_j - 1024q)*vis,
pipelined one pair ahead of the matmuls that consume it. Keypoint collisions
sum in PSUM naturally.

The attention reduction z[s] = sum_o attn_w[o] combined[o,s] runs as a matmul
whose lhsT is attn_w replicated across 128 columns, so the PSUM result
[128, 512] already holds z broadcast across all partitions -- sigmoid and the
final elementwise multiply need no partition-broadcast step.

Matmuls run in bf16 (full PE rate, FWL weight loads, HAM warms up). The PE
reads the image as a TRUNCATED-bf16 strided view of the fp32 tiles (top two
bytes of each f32 via bitcast + stride-2 AP) -- no cast ops, no extra DMA.
The final multiply uses the original fp32 image tiles, so output error comes
only through `scores` (~1.3e-3 relative). Loads issue on the sync HWDGE ring
and stores on the scalar HWDGE ring (independent FIFOs).

Sharding: pure data parallelism, batch b -> NeuronCore b (weights replicated).
"""

import sys
from contextlib import ExitStack

import numpy as np

sys.path.insert(0, "/opt/trn_rl_repo")

import concourse.bacc as bacc
import concourse.bass as bass
import concourse.mybir as mybir
import concourse.tile as tile
from concourse.bass_utils import run_bass_kernel_spmd

F32 = mybir.dt.float32
BF16 = mybir.dt.bfloat16
I32 = mybir.dt.int32
AF = mybir.ActivationFunctionType
OP = mybir.AluOpType

B, C, H, W, K = 8, 256, 128, 128, 19
S = H * W                  # 16384 pixels
ST = 512                   # pixel tile (one PSUM bank)
NT = S // ST               # 32 tiles
_CACHE: dict = {}


def _emit(tc: tile.TileContext, io: dict):
    nc = tc.nc
    img, kp, wt, mt, bias, arep, ab, out = (
        io["img"], io["kp"], io["wt"], io["mt"],
        io["bias"], io["arep"], io["ab"], io["out"],
    )
    with ExitStack() as ctx:
        consts = ctx.enter_context(tc.tile_pool(name="consts", bufs=1))
        small = ctx.enter_context(tc.tile_pool(name="small", bufs=1))
        imgp = ctx.enter_context(tc.tile_pool(name="imgp", bufs=6))
        combp = ctx.enter_context(tc.tile_pool(name="combp", bufs=6))
        scorep = ctx.enter_context(tc.tile_pool(name="scorep", bufs=4))
        outp = ctx.enter_context(tc.tile_pool(name="outp", bufs=4))
        psum = ctx.enter_context(tc.tile_pool(name="psum", bufs=2, space="PSUM"))
        ohp = ctx.enter_context(tc.tile_pool(name="ohp", bufs=3))

        # ---- constants into SBUF (weights pre-cast to bf16 on host) ----
        wt0 = consts.tile([128, C], BF16)          # W^T rows c=0..127
        wt1 = consts.tile([128, C], BF16)          # W^T rows c=128..255
        nc.sync.dma_start(wt0[:], wt[0:128, :])
        nc.sync.dma_start(wt1[:], wt[128:256, :])
        mts = consts.tile([K, C], BF16)            # M^T [19, 256]
        nc.sync.dma_start(mts[:], mt[:, :])
        kpt = small.tile([K, 3], F32)
        nc.scalar.dma_start(kpt[:], kp[:, :])
        ar0 = consts.tile([128, 128], BF16)        # attn_w replicated, o=0..127
        ar1 = consts.tile([128, 128], BF16)
        nc.scalar.dma_start(ar0[:], arep[0:128, :])
        nc.scalar.dma_start(ar1[:], arep[128:256, :])
        b0 = consts.tile([128, 1], F32)
        b1 = consts.tile([128, 1], F32)
        nc.scalar.dma_start(b0[:], bias[0:128, :])
        nc.scalar.dma_start(b1[:], bias[128:256, :])
        abt = consts.tile([128, 1], F32)
        nc.scalar.dma_start(abt[:], ab[:, :])

        # ---- build one-hot [K, S] on device ----
        # index math (all [19,1], exact fp32; matches reference:
        # x = int(clip(kx/128, 0, 127)), s = y*128 + x)

        def floor_clipped(col):
            v = small.tile([K, 1], F32, name=f"v{col}")
            nc.vector.tensor_scalar(v[:], kpt[:, col:col + 1], 1.0 / 128.0, None, OP.mult)
            nc.vector.tensor_scalar(v[:], v[:], 127.0, 0.0, OP.min, OP.max)
            vi = small.tile([K, 1], I32, name=f"vi{col}")
            nc.vector.tensor_copy(vi[:], v[:])        # any rounding mode works:
            vf = small.tile([K, 1], F32, name=f"vf{col}")
            nc.vector.tensor_copy(vf[:], vi[:])       # fixed up below
            gt = small.tile([K, 1], F32, name=f"gt{col}")
            nc.vector.tensor_tensor(gt[:], vf[:], v[:], op=OP.is_gt)
            nc.vector.tensor_tensor(vf[:], vf[:], gt[:], op=OP.subtract)
            return vf

        xf = floor_clipped(0)
        yf = floor_clipped(1)
        sf = small.tile([K, 1], F32)                  # pixel index y*128+x
        nc.vector.tensor_scalar(sf[:], yf[:], 128.0, xf[:, 0:1], OP.mult, OP.add)
        vis = small.tile([K, 1], F32)                 # 1.0 where visible
        nc.vector.tensor_scalar(vis[:], kpt[:, 2:3], 0.0, None, OP.is_gt)
        ioti = small.tile([K, 1024], I32)             # 0..1023 along free dim
        nc.gpsimd.iota(ioti[:], pattern=[[1, 1024]], base=0, channel_multiplier=0)
        iotf = small.tile([K, 1024], F32)
        nc.vector.tensor_copy(iotf[:], ioti[:])

        # one-hot chunk for pair q (1024 px): (iota == s - 1024q) * vis, one
        # fused DVE op per chunk; emitted one pair ahead of its consumers.
        def make_chunk(q):
            cv = small.tile([K, 1], F32, name=f"cv{q}")
            nc.vector.tensor_scalar(cv[:], sf[:], float(1024 * q), None, OP.subtract)
            oc = ohp.tile([K, 1024], BF16, tag="oh")
            nc.vector.tensor_scalar(oc[:], iotf[:], cv[:, 0:1], vis[:, 0:1],
                                    OP.is_equal, OP.mult)
            return oc

        # ---- main pixel loop: pairs of 512-px tiles (1024 px per DMA) ----
        # Attention matmuls + sigmoid + final mul run TWO pairs BEHIND the
        # main matmuls, so the PE stream never waits on a tanh issued in the
        # same iteration (keeps PE dense -> HAM stays warm).
        PT = 2 * ST
        NP = NT // 2
        from collections import deque
        pending = deque()          # attn stage runs TWO pairs behind
        DEPTH = 2
        next_chunk = make_chunk(0)

        def drain(dfr):
            sc, dim0, dim1, dslp, halves = dfr
            (dcb0a, dcb1a, dhs_a), (dcb0b, dcb1b, dhs_b) = halves
            pza = psum.tile([128, ST], F32, tag="psz", name="pza")
            pzb = psum.tile([128, ST], F32, tag="psz", name="pzb")
            nc.tensor.matmul(out=pza[:], lhsT=ar0[:], rhs=dcb0a[:], start=True, stop=False)
            nc.tensor.matmul(out=pzb[:], lhsT=ar0[:], rhs=dcb0b[:], start=True, stop=False)
            nc.tensor.matmul(out=pza[:], lhsT=ar1[:], rhs=dcb1a[:], start=False, stop=True)
            nc.tensor.matmul(out=pzb[:], lhsT=ar1[:], rhs=dcb1b[:], start=False, stop=True)
            nc.scalar.activation(sc[:, dhs_a], pza[:], AF.Sigmoid, bias=abt[:, 0:1])
            nc.scalar.activation(sc[:, dhs_b], pzb[:], AF.Sigmoid, bias=abt[:, 0:1])
            o0 = outp.tile([128, PT], F32, tag="o0")
            o1 = outp.tile([128, PT], F32, tag="o1")
            nc.vector.tensor_mul(o0[:], dim0[:], sc[:])
            nc.vector.tensor_mul(o1[:], dim1[:], sc[:])
            nc.scalar.dma_start(out[0:128, dslp], o0[:])
            nc.scalar.dma_start(out[128:256, dslp], o1[:])

        for p in range(NP):
            slp = bass.ts(p, PT)
            im0 = imgp.tile([128, PT], F32, tag="im0")
            im1 = imgp.tile([128, PT], F32, tag="im1")
            nc.sync.dma_start(im0[:], img[0:128, slp])
            nc.sync.dma_start(im1[:], img[128:256, slp])
            # truncated-bf16 views of the fp32 tiles (top 2 bytes of each f32)
            ib0 = im0[:].bitcast(BF16)[:, 1::2]
            ib1 = im1[:].bitcast(BF16)[:, 1::2]

            sc = scorep.tile([128, PT], F32, tag="sc")
            oh = next_chunk
            if p + 1 < NP:
                next_chunk = make_chunk(p + 1)
            if len(pending) >= DEPTH:
                drain(pending.popleft())
            if p == NP - 1 and pending:
                drain(pending.popleft())   # pull the tail stage into the loop
            # same stationary weight used for both halves back-to-back
            hs0, hs1 = bass.ts(0, ST), bass.ts(1, ST)
            pA0 = psum.tile([128, ST], F32, tag="ps0", bufs=3)
            pB0 = psum.tile([128, ST], F32, tag="ps0", bufs=3, name="pB0")
            pA1 = psum.tile([128, ST], F32, tag="ps1", bufs=3)
            pB1 = psum.tile([128, ST], F32, tag="ps1", bufs=3, name="pB1")
            nc.tensor.matmul(out=pA0[:], lhsT=wt0[:, 0:128], rhs=ib0[:, hs0], start=True, stop=False)
            nc.tensor.matmul(out=pB0[:], lhsT=wt0[:, 0:128], rhs=ib0[:, hs1], start=True, stop=False)
            nc.tensor.matmul(out=pA0[:], lhsT=wt1[:, 0:128], rhs=ib1[:, hs0], start=False, stop=False)
            nc.tensor.matmul(out=pB0[:], lhsT=wt1[:, 0:128], rhs=ib1[:, hs1], start=False, stop=False)
            nc.tensor.matmul(out=pA0[:], lhsT=mts[:, 0:128], rhs=oh[:, hs0], start=False, stop=True)
            nc.tensor.matmul(out=pB0[:], lhsT=mts[:, 0:128], rhs=oh[:, hs1], start=False, stop=True)
            nc.tensor.matmul(out=pA1[:], lhsT=wt0[:, 128:256], rhs=ib0[:, hs0], start=True, stop=False)
            nc.tensor.matmul(out=pB1[:], lhsT=wt0[:, 128:256], rhs=ib0[:, hs1], start=True, stop=False)
            nc.tensor.matmul(out=pA1[:], lhsT=wt1[:, 128:256], rhs=ib1[:, hs0], start=False, stop=False)
            nc.tensor.matmul(out=pB1[:], lhsT=wt1[:, 128:256], rhs=ib1[:, hs1], start=False, stop=False)
            nc.tensor.matmul(out=pA1[:], lhsT=mts[:, 128:256], rhs=oh[:, hs0], start=False, stop=True)
            nc.tensor.matmul(out=pB1[:], lhsT=mts[:, 128:256], rhs=oh[:, hs1], start=False, stop=True)

            halves = []
            for h, (q0, q1) in enumerate(((pA0, pA1), (pB0, pB1))):
                cb0 = combp.tile([128, ST], BF16, tag="cb0")
                cb1 = combp.tile([128, ST], BF16, tag="cb1")
                nc.scalar.activation(cb0[:], q0[:], AF.Tanh, bias=b0[:, 0:1])
                nc.scalar.activation(cb1[:], q1[:], AF.Tanh, bias=b1[:, 0:1])
                halves.append((cb0, cb1, bass.ts(h, ST)))

            pending.append((sc, im0, im1, slp, halves))

        while pending:
            drain(pending.popleft())

def _build():
    if "nc" in _CACHE:
        return _CACHE["nc"]
    nc = bacc.Bacc("TRN2", target_bir_lowering=False, debug=False)
    io = {
        "img": nc.dram_tensor("img", [C, S], F32, kind="ExternalInput").ap(),
        "kp": nc.dram_tensor("kp", [K, 3], F32, kind="ExternalInput").ap(),
        "wt": nc.dram_tensor("wt", [C, C], BF16, kind="ExternalInput").ap(),
        "mt": nc.dram_tensor("mt", [K, C], BF16, kind="ExternalInput").ap(),
        "bias": nc.dram_tensor("bias", [C, 1], F32, kind="ExternalInput").ap(),
        "arep": nc.dram_tensor("arep", [C, 128], BF16, kind="ExternalInput").ap(),
        "ab": nc.dram_tensor("ab", [128, 1], F32, kind="ExternalInput").ap(),
        "out": nc.dram_tensor("out", [C, S], F32, kind="ExternalOutput").ap(),
    }
    with tile.TileContext(nc) as tc:
        _emit(tc, io)
    nc.compile()
    _CACHE["nc"] = nc
    return nc


def _in_maps(image_features, keypoint_features, img_fc_w, img_fc_b,
             kp_proj_w, kp_proj_b, kp_fc_w, kp_fc_b, attn_fc_w, attn_fc_b):
    import ml_dtypes

    f = lambda a: np.ascontiguousarray(np.asarray(a, dtype=np.float32))
    bf = lambda a: np.ascontiguousarray(np.asarray(a, dtype=np.float32).astype(ml_dtypes.bfloat16))
    img_fc_w, img_fc_b = f(img_fc_w), f(img_fc_b)
    kp_proj_w, kp_proj_b = f(kp_proj_w), f(kp_proj_b)
    kp_fc_w, kp_fc_b = f(kp_fc_w), f(kp_fc_b)
    attn_fc_w, attn_fc_b = f(attn_fc_w), f(attn_fc_b)

    wt = bf(img_fc_w.T)                                         # [C, C]
    mt = bf((kp_fc_w @ kp_proj_w).T)                            # [K, C]
    bias = f((img_fc_b + kp_fc_w @ kp_proj_b + kp_fc_b).reshape(C, 1))
    arep = bf(np.repeat(attn_fc_w.reshape(C, 1), 128, axis=1))
    ab = np.full((128, 1), float(attn_fc_b.reshape(-1)[0]), np.float32)

    imgs = f(image_features).reshape(B, C, S)
    kps = f(keypoint_features)
    return [
        {
            "img": np.ascontiguousarray(imgs[b]),
            "kp": np.ascontiguousarray(kps[b]),
            "wt": wt, "mt": mt, "bias": bias, "arep": arep, "ab": ab,
        }
        for b in range(B)
    ]


def _run(in_maps, trace=False, tmpdir=None):
    nc = _build()
    return run_bass_kernel_spmd(
        nc, in_maps, core_ids=list(range(B)), trace=trace, tmpdir=tmpdir
    )


def kernel(**inputs) -> np.ndarray:
    res = _run(_in_maps(**inputs))
    return np.stack([res.results[b]["out"].reshape(C, H, W) for b in range(B)])


def _enable_axon_ntff_hook():
    """Recreate the missing antenv.axon_hooks module and register the NTFF
    profile hook (what trn_boot would do if the image shipped axon_hooks).
    Local profiling only; kernel() never calls this."""
    import types

    if "antenv.axon_hooks" in sys.modules:
        return
    mod = types.ModuleType("antenv.axon_hooks")
    state = {"hook": None}
    mod.set_axon_ntff_profile_hook = lambda h: state.__setitem__("hook", h)
    mod.get_axon_ntff_profile_hook = lambda: state["hook"]
    sys.modules["antenv.axon_hooks"] = mod
    import antenv

    antenv.axon_hooks = mod
    from trn_agent_boot.trn_boot import _ntff_profile_via_ctypes

    mod.set_axon_ntff_profile_hook(_ntff_profile_via_ctypes("/opt/axon/libaxon_pjrt.so"))
    # keep artifacts local -- no bucket in this container
    import concourse.bass_utils as bu

    bu.upload_artifacts = lambda tmpdir: tmpdir


def kernel_traced(**inputs):
    """Like kernel() but profiles: returns (out, exec_time_ns, tmpdir)."""
    import tempfile

    _enable_axon_ntff_hook()
    tmpdir = tempfile.mkdtemp(prefix="bass_trace_")
    res = _run(_in_maps(**inputs), trace=True, tmpdir=tmpdir)
    out = np.stack([res.results[b]["out"].reshape(C, H, W) for b in range(B)])
    return out, res.exec_time_ns, tmpdir
